# revision 48
# baseline (speedup 1.0000x reference)
"""Multi-head attention (B=2, T=2048, d_model=1024, 16 heads) on 8 TRN2 cores.

Sharding: data parallel over batch (2) x tensor parallel over heads (4 groups
of 4 heads). core = b*4 + g. Each core computes its 4 heads' attention
probabilities (written in full) plus its partial contribution to the output
projection; the host sums the 4 partials per batch.

Per-core device kernel (matmuls in float32r, ~1.5e-4 rel err; inputs are
DMA'd straight into f32r tiles):
  phase A: QT/KT [256,2048] (transposed layouts) and V [2048,256] from
           host-pre-transposed x/w; x_kv is streamed once and feeds both.
  phase B: S-side: S = Q.K^T via row-packed K=64 matmul pairs; one ACT pass
           exp(S/8) with accum_out row sums; DVE per-partition normalize ->
           attn output. S^T-side: S^T computed directly on the PE (no 16M-
           element transposes anywhere), exp'd, consumed by O += P~^T.T @ V;
           1/l normalization folded in later via a broadcast reciprocal
           matrix. The two chains are interleaved to keep ACT (the
           bottleneck engine) saturated; the final 8 S^T parts get a
           double-buffered PSUM pool (banks freed by the S-side) and are
           overlapped with the pair-0 out-projection.
  phase C: out_partial = O_norm @ w_o_slice^T (pair 1 + precomputed pair 0).
"""

import sys

import numpy as np

try:
    import concourse.bacc as bacc  # noqa: F401
except ImportError:  # harness may run from a bare directory
    sys.path.insert(0, "/opt/trn_rl_repo")

import concourse.bacc as bacc
import concourse.mybir as mybir
import concourse.tile as tile
from concourse.bass_utils import run_bass_kernel_spmd

F32 = mybir.dt.float32
F32R = mybir.dt.float32r
AF = mybir.ActivationFunctionType

B = 2
T = 2048
C = 1024           # d_model
NH = 16            # total heads
DH = 64            # head dim
HPC = 4            # heads per core
DPC = HPC * DH     # 256, d' slice per core
NCORES = 8
SCALE = 0.125      # 1/sqrt(DH)

NCC = C // 128     # 8 contraction chunks
NQC = T // 128     # 16 q chunks
NQB = T // 512     # 4 q blocks
NKC = T // 128     # 16 k chunks

_CACHE = {}


def _build_nc():
    nc = bacc.Bacc("TRN2", target_bir_lowering=False, debug=False)

    xq_t = nc.declare_dram_parameter("xq_t", [C, T], F32, isOutput=False)
    xkv_t = nc.declare_dram_parameter("xkv_t", [C, T], F32, isOutput=False)
    wq_t = nc.declare_dram_parameter("wq_t", [C, DPC], F32, isOutput=False)
    wk_t = nc.declare_dram_parameter("wk_t", [C, DPC], F32, isOutput=False)
    wv_t = nc.declare_dram_parameter("wv_t", [C, DPC], F32, isOutput=False)
    wo_t = nc.declare_dram_parameter("wo_t", [DPC, C], F32, isOutput=False)
    attn4 = nc.declare_dram_parameter("attn4", [HPC, T, T], F32, isOutput=True)
    outp = nc.declare_dram_parameter("outp", [T, C], F32, isOutput=True)

    with tile.TileContext(nc) as tc:
        with tc.tile_pool(name="persist", bufs=1) as pp, \
             tc.tile_pool(name="findr", bufs=1, space="DRAM") as findr, \
             tc.tile_pool(name="otps", bufs=1, space="PSUM") as otps:
            QT = [pp.tile([128, T], F32R, tag=f"qt{d}", name=f"qt{d}") for d in range(2)]
            KT = [pp.tile([128, T], F32R, tag=f"kt{d}", name=f"kt{d}") for d in range(2)]
            V = [pp.tile([128, DPC], F32R, tag=f"v{t}", name=f"v{t}") for t in range(NKC)]
            OT = [pp.tile([128, T], F32R, tag=f"ot{p}", name=f"ot{p}") for p in range(2)]
            WO = [pp.tile([128, C], F32R, tag=f"wo{p}", name=f"wo{p}") for p in range(2)]
            l_all = pp.tile([128, 128], F32, tag="l_all")
            rl_store = pp.tile([128, 64], F32, tag="rl_store")

            def emit_st_part(stpool, ptpool, p, qb, kcs, ot, wide=False):
                # S^T + exp + O accumulation for k chunks `kcs` of q block qb.
                # wide=True packs 2 k-chunks per PSUM slot / FD=2048 exp call
                # (halves the ACT per-call overhead on the S^T side).
                group = 2 if wide else 1
                for g0 in range(0, len(kcs), group):
                    kcg = kcs[g0:g0 + group]
                    st = stpool.tile([128, 1024 * group], F32, tag="st", name="st")
                    for ki, kc in enumerate(kcg):
                        for hl in range(2):
                            lo, hi = hl * 64, (hl + 1) * 64
                            tpos = None if hl == 0 else (64, 0)
                            nc.tensor.matmul(
                                st[:, ki * 1024 + hl * 512:ki * 1024 + (hl + 1) * 512],
                                KT[p][lo:hi, kc * 128:(kc + 1) * 128],
                                QT[p][lo:hi, qb * 512:(qb + 1) * 512],
                                start=True, stop=True, tile_position=tpos)
                    pt = ptpool.tile([128, 1024 * group], F32R, tag="pt", name="pt")
                    nc.scalar.activation(pt[:], st[:], AF.Exp, scale=SCALE)
                    for ki, kc in enumerate(kcg):
                        nc.tensor.matmul(
                            ot[0][:], V[kc][:, p * 128:p * 128 + 64],
                            pt[:, ki * 1024:ki * 1024 + 512],
                            start=(kc == 0), stop=(kc == NKC - 1))
                        nc.tensor.matmul(
                            ot[1][:], V[kc][:, p * 128 + 64:p * 128 + 128],
                            pt[:, ki * 1024 + 512:(ki + 1) * 1024],
                            start=(kc == 0), stop=(kc == NKC - 1))
                if kcs[-1] == NKC - 1:
                    nc.vector.tensor_copy(OT[p][0:64, qb * 512:(qb + 1) * 512], ot[0][:])
                    nc.vector.tensor_copy(OT[p][64:128, qb * 512:(qb + 1) * 512], ot[1][:])

            def emit_rl_chain(finpool, p):
                # reciprocal row matrix for pair p, then normalize OT[p]
                lrT = finpool.tile([32, 128], F32, tag=f"lrT{p}", name=f"lrT{p}")
                for j in range(4):
                    nc.vector.transpose(
                        lrT[:, j * 32:(j + 1) * 32],
                        rl_store[j * 32:(j + 1) * 32, p * 32:(p + 1) * 32])
                lr_dram = findr.tile([32, 128], F32, tag=f"lrd{p}", name=f"lrd{p}")
                nc.sync.dma_start(lr_dram[:], lrT[:])
                rlmat = finpool.tile([128, T], F32, tag=f"rlmat{p}", name=f"rlmat{p}")
                for hl in range(2):
                    nc.sync.dma_start(
                        rlmat[hl * 64:(hl + 1) * 64, :].rearrange(
                            "p (c i) -> p c i", c=16),
                        lr_dram[hl * 16:(hl + 1) * 16, :].partition_broadcast(64))
                nc.vector.tensor_mul(OT[p][:], OT[p][:], rlmat[:])

            with tc.tile_pool(name="pq", bufs=7) as pq_pool, \
                 tc.tile_pool(name="sps", bufs=2, space="PSUM") as sps:

                def emit_s_side(p, qc):
                    # attn probabilities + row sums, heads (2p, 2p+1), q chunk qc
                    pq = [pq_pool.tile([128, T], F32, tag="pq", name="pq")
                          for _ in range(2)]
                    base = qc * 8 + p * 4
                    for kb in range(2):
                        s_ps = [sps.tile([128, 1024], F32, tag="s", name="s")
                                for _ in range(2)]
                        for hl in range(2):
                            lo, hi = hl * 64, (hl + 1) * 64
                            tpos = None if hl == 0 else (64, 0)
                            for n in range(2):
                                nc.tensor.matmul(
                                    s_ps[hl][:, n * 512:(n + 1) * 512],
                                    QT[p][lo:hi, qc * 128:(qc + 1) * 128],
                                    KT[p][lo:hi, kb * 1024 + n * 512:kb * 1024 + (n + 1) * 512],
                                    start=True, stop=True, tile_position=tpos)
                            nc.scalar.activation(
                                pq[hl][:, kb * 1024:(kb + 1) * 1024], s_ps[hl][:],
                                AF.Exp, scale=SCALE,
                                accum_out=l_all[:, base + hl * 2 + kb:base + hl * 2 + kb + 1])
                    lsum = pq_pool.tile([128, 2], F32, tag="lsum")
                    nc.vector.tensor_add(lsum[:], l_all[:, base:base + 4:2],
                                         l_all[:, base + 1:base + 4:2])
                    c0 = (2 * p) * 16 + qc
                    nc.vector.reciprocal(rl_store[:, c0:c0 + 17:16], lsum[:])
                    for hl in range(2):
                        h = 2 * p + hl
                        nc.vector.tensor_scalar_mul(
                            pq[hl][:], pq[hl][:],
                            rl_store[:, h * 16 + qc:h * 16 + qc + 1])
                        nc.sync.dma_start(
                            attn4[h, qc * 128:(qc + 1) * 128, :], pq[hl][:])

                # ---------------- phase A ----------------
                with tc.tile_pool(name="wpool", bufs=1) as wp, \
                     tc.tile_pool(name="xkvs", bufs=15) as xkvs, \
                     tc.tile_pool(name="xqs", bufs=11) as xqs, \
                     tc.tile_pool(name="pps", bufs=2, space="PSUM") as pps:
                    def load_w(wdram, nm):
                        tiles = []
                        for cc in range(NCC):
                            wr = wp.tile([128, DPC], F32R, tag=f"{nm}{cc}",
                                         name=f"{nm}{cc}")
                            nc.sync.dma_start(
                                wr[:], wdram[cc * 128:(cc + 1) * 128, :].bitcast(F32R))
                            tiles.append(wr)
                        return tiles

                    WK_t = load_w(wk_t, "wk")

                    # stream x_kv once per q-block; feeds KT (both halves) and V
                    WV_t = None
                    for qb in range(NQB):
                        XK = []
                        for cc in range(NCC):
                            xr = xkvs.tile([128, 512], F32R, tag="xkv", name="xkv")
                            nc.sync.dma_start(
                                xr[:],
                                xkv_t[cc * 128:(cc + 1) * 128,
                                      qb * 512:(qb + 1) * 512].bitcast(F32R))
                            XK.append(xr)
                        for dt in range(2):
                            ps_kt = pps.tile([128, 512], F32, tag="ps", name="ps_kt")
                            for cc in range(NCC):
                                nc.tensor.matmul(
                                    ps_kt[:], WK_t[cc][:, dt * 128:(dt + 1) * 128],
                                    XK[cc][:],
                                    start=(cc == 0), stop=(cc == NCC - 1))
                            nc.vector.tensor_copy(
                                KT[dt][:, qb * 512:(qb + 1) * 512], ps_kt[:])
                        if WV_t is None:
                            WV_t = load_w(wv_t, "wv")
                        for j in range(4):
                            tck = qb * 4 + j
                            ps_v = pps.tile([128, DPC], F32, tag="ps", name="ps_v")
                            for cc in range(NCC):
                                nc.tensor.matmul(
                                    ps_v[:], XK[cc][:, j * 128:(j + 1) * 128],
                                    WV_t[cc][:],
                                    start=(cc == 0), stop=(cc == NCC - 1))
                            nc.vector.tensor_copy(V[tck][:], ps_v[:])

                    WQ_t = load_w(wq_t, "wq")
                    for p in range(2):
                        nc.sync.dma_start(
                            WO[p][:], wo_t[p * 128:(p + 1) * 128, :].bitcast(F32R))

                    # stream x_q once per q-block; 2 S units overlap per block
                    for qb in range(NQB):
                        XQ = []
                        for cc in range(NCC):
                            xr = xqs.tile([128, 512], F32R, tag="xq", name="xq")
                            nc.sync.dma_start(
                                xr[:],
                                xq_t[cc * 128:(cc + 1) * 128,
                                     qb * 512:(qb + 1) * 512].bitcast(F32R))
                            XQ.append(xr)
                        for dt in range(2):
                            ps_qt = pps.tile([128, 512], F32, tag="ps", name="ps_qt")
                            for cc in range(NCC):
                                nc.tensor.matmul(
                                    ps_qt[:], WQ_t[cc][:, dt * 128:(dt + 1) * 128],
                                    XQ[cc][:],
                                    start=(cc == 0), stop=(cc == NCC - 1))
                            nc.vector.tensor_copy(
                                QT[dt][:, qb * 512:(qb + 1) * 512], ps_qt[:])
                        emit_s_side(0, 2 * qb)
                        emit_s_side(0, 2 * qb + 1)

                # ------------- phase B part 1: ST parts 0-23 + 24 S units -------------
                with tc.tile_pool(name="stps", bufs=1, space="PSUM") as stps, \
                     tc.tile_pool(name="pt1", bufs=6) as pt1, \
                     tc.tile_pool(name="fin1", bufs=1) as fin1:
                    s_units = [(0, qc) for qc in range(8, NQC)] + \
                              [(1, qc) for qc in range(NQC)]
                    s_idx = 0
                    ot = None
                    for gi in range(24):
                        p, i = divmod(gi, 16)
                        qb, part = divmod(i, 4)
                        if part == 0:
                            ot = [otps.tile([64, 512], F32, tag="otA", name="otA"),
                                  otps.tile([64, 512], F32, tag="otB", name="otB")]
                        emit_st_part(stps, pt1, p, qb,
                                     list(range(part * 4, part * 4 + 4)), ot)
                        if s_idx < len(s_units):
                            sp, sqc = s_units[s_idx]
                            s_idx += 1
                            emit_s_side(sp, sqc)
                        if gi == 15:
                            emit_rl_chain(fin1, 0)
                    assert s_idx == len(s_units)

            # ------------- phase B part 2 + phase C -------------
            # Last 8 ST parts run double-buffered while completed q-blocks of
            # OT[1] are normalized per-block and consumed by full
            # out-projection units (both pairs), draining the tail early.
            with tc.tile_pool(name="fin2", bufs=1) as fin2, \
                 tc.tile_pool(name="osb2", bufs=8) as osb2:

                def emit_rl_prep(finpool, p):
                    # build the reciprocal row matrix for pair p
                    lrT = finpool.tile([32, 128], F32, tag=f"lrT{p}", name=f"lrT{p}")
                    for j in range(4):
                        nc.vector.transpose(
                            lrT[:, j * 32:(j + 1) * 32],
                            rl_store[j * 32:(j + 1) * 32, p * 32:(p + 1) * 32])
                    lr_dram = findr.tile([32, 128], F32, tag=f"lrd{p}", name=f"lrd{p}")
                    nc.sync.dma_start(lr_dram[:], lrT[:])
                    rlmat = finpool.tile([128, T], F32, tag=f"rlmat{p}", name=f"rlmat{p}")
                    for hl in range(2):
                        nc.sync.dma_start(
                            rlmat[hl * 64:(hl + 1) * 64, :].rearrange(
                                "p (c i) -> p c i", c=16),
                            lr_dram[hl * 16:(hl + 1) * 16, :].partition_broadcast(64))
                    return rlmat

                def emit_ot_mul(p, rlmat, qb):
                    sl = slice(qb * 512, (qb + 1) * 512)
                    nc.vector.tensor_mul(OT[p][:, sl], OT[p][:, sl], rlmat[:, sl])

                def emit_out_unit(pool, tck, eh):
                    op_ps = pool.tile([128, 512], F32, tag="op", name="op")
                    for p in range(2):
                        nc.tensor.matmul(
                            op_ps[:], OT[p][:, tck * 128:(tck + 1) * 128],
                            WO[p][:, eh * 512:(eh + 1) * 512],
                            start=(p == 0), stop=(p == 1))
                    o_sb = osb2.tile([128, 512], F32, tag="osb", name="osb")
                    if (tck + eh) % 2 == 0:
                        nc.scalar.copy(o_sb[:], op_ps[:])
                    else:
                        nc.vector.tensor_copy(o_sb[:], op_ps[:])
                    nc.sync.dma_start(
                        outp[tck * 128:(tck + 1) * 128, eh * 512:(eh + 1) * 512],
                        o_sb[:])

                with tc.tile_pool(name="stps2", bufs=2, space="PSUM") as stps2, \
                     tc.tile_pool(name="ops", bufs=2, space="PSUM") as opsp, \
                     tc.tile_pool(name="pt2", bufs=4) as pt2:
                    rlmat1 = emit_rl_prep(fin2, 1)
                    emit_ot_mul(1, rlmat1, 0)   # OT[1] qb 0/1 drained in part 1
                    emit_ot_mul(1, rlmat1, 1)
                    units = [(tck, eh) for tck in range(8) for eh in range(2)]
                    ui = 0
                    ot = None
                    for gi in range(24, 32):
                        p, i = divmod(gi, 16)
                        qb, part = divmod(i, 4)
                        if part == 0:
                            ot = [otps.tile([64, 512], F32, tag="otA", name="otA"),
                                  otps.tile([64, 512], F32, tag="otB", name="otB")]
                        emit_st_part(stps2, pt2, p, qb,
                                     list(range(part * 4, part * 4 + 4)), ot)
                        for _ in range(2):
                            emit_out_unit(opsp, *units[ui])
                            ui += 1
                        if gi == 27:
                            emit_ot_mul(1, rlmat1, 2)
                    emit_ot_mul(1, rlmat1, 3)

                # remaining out tiles (t chunks 8-15) after stps2 frees banks
                with tc.tile_pool(name="ops2", bufs=6, space="PSUM") as opsp2:
                    for tck in range(8, NQC):
                        for eh in range(2):
                            emit_out_unit(opsp2, tck, eh)

    nc.compile()
    return nc


def _get_nc():
    if "nc" not in _CACHE:
        _CACHE["nc"] = _build_nc()
    return _CACHE["nc"]


def kernel(x_q, x_kv, w_q, w_k, w_v, w_o):
    x_q = np.asarray(x_q, dtype=np.float32)
    x_kv = np.asarray(x_kv, dtype=np.float32)
    w_q = np.asarray(w_q, dtype=np.float32)
    w_k = np.asarray(w_k, dtype=np.float32)
    w_v = np.asarray(w_v, dtype=np.float32)
    w_o = np.asarray(w_o, dtype=np.float32)

    nc = _get_nc()

    in_maps = []
    for core in range(NCORES):
        b, g = divmod(core, 4)
        sl = slice(g * DPC, (g + 1) * DPC)
        in_maps.append({
            "xq_t": np.ascontiguousarray(x_q[b].T),
            "xkv_t": np.ascontiguousarray(x_kv[b].T),
            "wq_t": np.ascontiguousarray(w_q[sl, :].T),
            "wk_t": np.ascontiguousarray(w_k[sl, :].T),
            "wv_t": np.ascontiguousarray(w_v[sl, :].T),
            "wo_t": np.ascontiguousarray(w_o[:, sl].T),
        })

    res = run_bass_kernel_spmd(nc, in_maps, list(range(NCORES)))

    attn = np.empty((B, NH, T, T), np.float32)
    out = np.zeros((B, T, C), np.float32)
    for core in range(NCORES):
        b, g = divmod(core, 4)
        r = res.results[core]
        attn[b, g * HPC:(g + 1) * HPC] = r["attn4"]
        out[b] += r["outp"]
    return out, attn
